# revision 20
# baseline (speedup 1.0000x reference)
"""BackgroundNoiseLayer kernel for 8 trn2 NeuronCores.

Math: out[0, t, n] = sum_k W[n, k] * rest[t, k], where W [60000, 100] is
scatter-added from COO (v1 block rows 0..49999, lm block rows 50000..59999)
and the output feature axis is the concat of the two blocks.

Strategy (per sharding hint): densify the tiny sparse matrix host-side
(240k nnz -> dense W, ~0.002% of the matmul FLOPs), shard the post-synaptic
feature axis across the 8 cores (7500 features each), and run a dense
[1000,100] @ [100,7500] matmul per core. rest is tiny and replicated. Each
core writes its own contiguous output slice; concat on host is the no-op
gather.

Device-side choices (from profiling):
- Shapes zero-padded to DMA-friendly sizes (K 100->112, rows 1000->1024,
  features/core 7500->7680=15*512): DMAs whose partition count is not a
  multiple of 16 measured ~2.7x slower (125p: 123 GB/s vs 128p: 339 GB/s;
  112p is fast and wastes less pad than 128), and 512-f32 matmul chunks
  land exactly on PSUM bank boundaries.
- fp32 matmul runs at 1/4 bf16 rate on the PE (walrus splits each into two
  half-rate passes). Instead W is decomposed into 3 bf16 planes
  (hi/mid/lo = 24 mantissa bits, i.e. fp32-exact) accumulated in fp32 PSUM:
  3 full-rate bf16 matmuls beat 1 fp32 matmul by ~25%. rest (Poisson spike
  counts, small integers) is bf16-exact; a host-side sparse correction
  covers any value that is not.
- in-DMAs ride the gpsimd (SWDGE) ring, out-DMAs the sync (HWDGE) ring, so
  input loads don't head-of-line block the output stream.
"""

import os

import numpy as np

B, T = 1, 1000
NBKG = 100
NV1, NLM = 50000, 10000
NPOST = NV1 + NLM          # 60000
NCORES = 8
SHARD = NPOST // NCORES    # 7500 real features per core

KP = 112                   # padded contraction dim (zeros in rows 100..111)
ROWS = 1024                # padded time rows (zeros in 1000..1023)
OUTP = 7680                # padded features per core = 15 * 512
TBLK = 128                 # rows per block = full partition set
NT = ROWS // TBLK          # 8
MMN = 512                  # matmul free dim = exactly one fp32 PSUM bank
NCH = OUTP // MMN          # 15 chunks per row block
WCH = 1536                 # w chunk: 3 matmuls worth of columns
NW = OUTP // WCH           # 5
NMM = WCH // MMN           # 3
NPLANE = 3                 # bf16 planes of W (hi/mid/lo)

_compiled = None


def _build_module():
    import concourse.bacc as bacc
    import concourse.mybir as mybir
    import concourse.tile as tile

    f32 = mybir.dt.float32
    bf16 = mybir.dt.bfloat16
    nc = bacc.Bacc("TRN2", target_bir_lowering=False, debug=False)
    restT = nc.dram_tensor("restT", [KP, ROWS], bf16, kind="ExternalInput")
    wT3 = nc.dram_tensor("wT3", [NPLANE, KP, OUTP], bf16, kind="ExternalInput")
    out = nc.dram_tensor("out", [ROWS, SHARD], f32, kind="ExternalOutput")

    with tile.TileContext(nc) as tc:
        with (
            tc.tile_pool(name="inp", bufs=1) as inp,
            tc.tile_pool(name="stage", bufs=3) as stagep,
            tc.tile_pool(name="psum", bufs=8, space="PSUM") as psump,
        ):
            rest_sb = inp.tile([KP, ROWS], bf16, tag="rest")
            nc.sync.dma_start(rest_sb[:], restT[:])
            # one DMA per column-chunk carrying all 3 planes, so matmuls
            # start after ~1.2 MB instead of the full 5.9 MB; first chunks
            # ride the two HWDGE rings (fast first-byte, idle until the out
            # stream starts), the rest the SWDGE ring
            wT3_psw = wT3[:].rearrange("s p n -> p s n")
            in_engines = [nc.sync, nc.scalar, nc.gpsimd, nc.gpsimd, nc.gpsimd]
            w_sb = []
            for c in range(NW):
                wt = inp.tile([KP, NPLANE * WCH], bf16, tag=f"w{c}")
                in_engines[c].dma_start(
                    wt[:].rearrange("p (s w) -> p s w", s=NPLANE),
                    wT3_psw[:, :, c * WCH:(c + 1) * WCH],
                )
                w_sb.append(wt)

            copy_engines = [nc.vector.tensor_copy, nc.scalar.copy]
            cidx = 0

            def group(tb, cw):
                # one w-column group for row block tb: 3 bank chunks, each
                # 3 accumulated plane-matmuls + a PSUM copy into a small
                # per-(tb,cw) column tile, then one piece DMA whose only
                # dependency is exactly those 3 copies
                nonlocal cidx
                st = stagep.tile([TBLK, WCH], f32, tag=f"s{cw}",
                                 name=f"st{tb}_{cw}")
                lhsT = rest_sb[:, tb * TBLK:(tb + 1) * TBLK]
                wt = w_sb[cw]
                for m in range(NMM):
                    ps = psump.tile([TBLK, MMN], f32, tag="ps")
                    off = m * MMN
                    for s in range(NPLANE):
                        nc.tensor.matmul(
                            ps[:],
                            lhsT,
                            wt[:, s * WCH + off:s * WCH + off + MMN],
                            start=(s == 0),
                            stop=(s == NPLANE - 1),
                        )
                    copy_engines[cidx % 2](st[:, off:off + MMN], ps[:])
                    cidx += 1
                lo = cw * WCH
                hi = min(lo + WCH, SHARD)
                nc.sync.dma_start(
                    out[tb * TBLK:(tb + 1) * TBLK, lo:hi], st[:, :hi - lo]
                )

            # Ramp blocks 0-2 chunk-major: as each w chunk lands the PE has
            # 3 blocks' worth of ready work and the column pieces stream out
            # while later chunks are still being read. Blocks 3-7 run
            # block-major, still streaming per-chunk pieces.
            RAMP = 3
            for cw in range(NW):
                for tb in range(RAMP):
                    group(tb, cw)
            for tb in range(RAMP, NT):
                for cw in range(NW):
                    group(tb, cw)

    nc.compile()
    return nc


def _densify(v1_weights, v1_rows, v1_cols, lm_weights, lm_rows, lm_cols):
    rows = np.concatenate([
        np.asarray(v1_rows).astype(np.int64),
        np.asarray(lm_rows).astype(np.int64) + NV1,
    ])
    cols = np.concatenate([
        np.asarray(v1_cols).astype(np.int64),
        np.asarray(lm_cols).astype(np.int64),
    ])
    w = np.concatenate([
        np.asarray(v1_weights, dtype=np.float32),
        np.asarray(lm_weights, dtype=np.float32),
    ])
    W = np.bincount(rows * NBKG + cols, weights=w, minlength=NPOST * NBKG)
    return W.astype(np.float32).reshape(NPOST, NBKG)


def kernel(rest, v1_weights, v1_rows, v1_cols, lm_weights, lm_rows, lm_cols):
    import ml_dtypes

    from concourse.bass_utils import run_bass_kernel_spmd

    bf16 = ml_dtypes.bfloat16

    global _compiled
    if _compiled is None:
        _compiled = _build_module()

    W = _densify(v1_weights, v1_rows, v1_cols, lm_weights, lm_rows, lm_cols)
    # 3-plane bf16 split: hi+mid+lo carries 24 mantissa bits == fp32-exact
    w_hi = W.astype(bf16)
    r1 = W - w_hi.astype(np.float32)
    w_mid = r1.astype(bf16)
    w_lo = (r1 - w_mid.astype(np.float32)).astype(bf16)

    rest32 = np.asarray(rest, np.float32)
    rest_b = rest32.astype(bf16)

    restT = np.zeros((KP, ROWS), bf16)
    restT[:NBKG, :B * T] = rest_b.T

    in_maps = []
    for c in range(NCORES):
        wpad = np.zeros((NPLANE, KP, OUTP), bf16)
        sl = slice(c * SHARD, (c + 1) * SHARD)
        wpad[0, :NBKG, :SHARD] = w_hi[sl].T
        wpad[1, :NBKG, :SHARD] = w_mid[sl].T
        wpad[2, :NBKG, :SHARD] = w_lo[sl].T
        in_maps.append({"restT": restT, "wT3": wpad})

    trace = bool(int(os.environ.get("KERNEL_TRACE", "0")))
    if trace:
        _install_ntff_shim()
    res = run_bass_kernel_spmd(
        _compiled, in_maps, core_ids=list(range(NCORES)), trace=trace
    )
    kernel.last_results = res
    full = np.concatenate(
        [res.results[c]["out"][:B * T, :] for c in range(NCORES)], axis=1
    )

    # sparse host correction for any rest value that bf16 can't represent
    # exactly (Poisson counts are small ints, so normally there are none)
    rest_err = rest32 - rest_b.astype(np.float32)
    if np.any(rest_err):
        ts, ks = np.nonzero(rest_err)
        for t, k in zip(ts, ks):
            full[t, :] += rest_err[t, k] * W[:, k]

    return full.reshape(B, T, NPOST)


def _install_ntff_shim():
    """The agent image's antenv lacks axon_hooks; register the NTFF profile
    hook by dlopening libaxon_pjrt.so directly (same path trn_boot uses)."""
    import sys
    import types

    if "antenv.axon_hooks" in sys.modules:
        return
    try:
        from trn_agent_boot.trn_boot import _ntff_profile_via_ctypes

        hook = _ntff_profile_via_ctypes("/opt/axon/libaxon_pjrt.so")
    except Exception:
        hook = None
    mod = types.ModuleType("antenv.axon_hooks")
    mod.get_axon_ntff_profile_hook = lambda: hook
    mod.set_axon_ntff_profile_hook = lambda h: None
    sys.modules["antenv.axon_hooks"] = mod


# revision 22
# speedup vs baseline: 1.0578x; 1.0578x over previous
"""BackgroundNoiseLayer kernel for 8 trn2 NeuronCores.

Math: out[0, t, n] = sum_k W[n, k] * rest[t, k], where W [60000, 100] is
scatter-added from COO (v1 block rows 0..49999, lm block rows 50000..59999)
and the output feature axis is the concat of the two blocks.

Strategy (per sharding hint): densify the tiny sparse matrix host-side
(240k nnz -> dense W, ~0.002% of the matmul FLOPs), shard the post-synaptic
feature axis across the 8 cores (7500 features each), and run a dense
[1000,100] @ [100,7500] matmul per core. rest is tiny and replicated. Each
core writes its own contiguous output slice; concat on host is the no-op
gather.

Device-side choices (from profiling):
- Shapes zero-padded to DMA-friendly sizes (K 100->112, rows 1000->1024,
  features/core 7500->7680=15*512): DMAs whose partition count is not a
  multiple of 16 measured ~2.7x slower (125p: 123 GB/s vs 128p: 339 GB/s;
  112p is fast and wastes less pad than 128), and 512-f32 matmul chunks
  land exactly on PSUM bank boundaries.
- fp32 matmul runs at 1/4 bf16 rate on the PE (walrus splits each into two
  half-rate passes). Instead W is decomposed into 3 bf16 planes
  (hi/mid/lo = 24 mantissa bits, i.e. fp32-exact) accumulated in fp32 PSUM:
  3 full-rate bf16 matmuls beat 1 fp32 matmul by ~25%. rest (Poisson spike
  counts, small integers) is bf16-exact; a host-side sparse correction
  covers any value that is not.
- in-DMAs ride the gpsimd (SWDGE) ring, out-DMAs the sync (HWDGE) ring, so
  input loads don't head-of-line block the output stream.
"""

import os

import numpy as np

B, T = 1, 1000
NBKG = 100
NV1, NLM = 50000, 10000
NPOST = NV1 + NLM          # 60000
NCORES = 8
SHARD = NPOST // NCORES    # 7500 real features per core

KP = 112                   # padded contraction dim (zeros in rows 100..111)
ROWS = 1024                # padded time rows (zeros in 1000..1023)
OUTP = 7680                # padded features per core = 15 * 512
TBLK = 128                 # rows per block = full partition set
NT = ROWS // TBLK          # 8
MMN = 512                  # matmul free dim = exactly one fp32 PSUM bank
NCH = OUTP // MMN          # 15 chunks per row block
WCH = 1536                 # w chunk: 3 matmuls worth of columns
NW = OUTP // WCH           # 5
NMM = WCH // MMN           # 3
NPLANE = 3                 # bf16 planes of W (hi/mid/lo)

_compiled = None


def _build_module():
    import concourse.bacc as bacc
    import concourse.mybir as mybir
    import concourse.tile as tile

    f32 = mybir.dt.float32
    bf16 = mybir.dt.bfloat16
    nc = bacc.Bacc("TRN2", target_bir_lowering=False, debug=False)
    restT = nc.dram_tensor("restT", [KP, ROWS], bf16, kind="ExternalInput")
    wT3 = nc.dram_tensor("wT3", [NPLANE, KP, OUTP], bf16, kind="ExternalInput")
    out = nc.dram_tensor("out", [ROWS, SHARD], f32, kind="ExternalOutput")

    with tile.TileContext(nc) as tc:
        with (
            tc.tile_pool(name="inp", bufs=1) as inp,
            tc.tile_pool(name="stage", bufs=3) as stagep,
            tc.tile_pool(name="psum", bufs=8, space="PSUM") as psump,
        ):
            rest_sb = inp.tile([KP, ROWS], bf16, tag="rest")
            nc.sync.dma_start(rest_sb[:], restT[:])
            # one DMA per column-chunk carrying all 3 planes, so matmuls
            # start after ~1.2 MB instead of the full 5.9 MB; first chunks
            # ride the two HWDGE rings (fast first-byte, idle until the out
            # stream starts), the rest the SWDGE ring
            wT3_psw = wT3[:].rearrange("s p n -> p s n")
            in_engines = [nc.sync, nc.scalar, nc.gpsimd, nc.gpsimd, nc.gpsimd]
            w_sb = []
            for c in range(NW):
                wt = inp.tile([KP, NPLANE * WCH], bf16, tag=f"w{c}")
                in_engines[c].dma_start(
                    wt[:].rearrange("p (s w) -> p s w", s=NPLANE),
                    wT3_psw[:, :, c * WCH:(c + 1) * WCH],
                )
                w_sb.append(wt)

            copy_engines = [nc.vector.tensor_copy, nc.scalar.copy]
            cidx = 0

            def mm_chunk(dst, dst_off, tb, cw, m):
                # 3 accumulated plane-matmuls for one 512-col bank chunk
                # + one PSUM->SBUF copy into dst at dst_off
                nonlocal cidx
                ps = psump.tile([TBLK, MMN], f32, tag="ps")
                lhsT = rest_sb[:, tb * TBLK:(tb + 1) * TBLK]
                wt = w_sb[cw]
                off = m * MMN
                for s in range(NPLANE):
                    nc.tensor.matmul(
                        ps[:],
                        lhsT,
                        wt[:, s * WCH + off:s * WCH + off + MMN],
                        start=(s == 0),
                        stop=(s == NPLANE - 1),
                    )
                copy_engines[cidx % 2](dst[:, dst_off:dst_off + MMN], ps[:])
                cidx += 1

            # Ramp blocks 0-1, chunk-major with per-(block, chunk) column
            # tiles: as each w chunk lands the PE has 2 blocks of ready
            # work, and each piece DMA depends on exactly its own 3 copies,
            # so the write stream starts while later chunks are still being
            # read instead of idling until a whole block is finished.
            RAMP = 2
            for cw in range(NW):
                lo = cw * WCH
                hi = min(lo + WCH, SHARD)
                for tb in range(RAMP):
                    st = stagep.tile([TBLK, WCH], f32, tag=f"s{cw}",
                                     name=f"st{tb}_{cw}", bufs=RAMP)
                    for m in range(NMM):
                        mm_chunk(st, m * MMN, tb, cw, m)
                    nc.sync.dma_start(
                        out[tb * TBLK:(tb + 1) * TBLK, lo:hi],
                        st[:, :hi - lo],
                    )

            # steady state blocks 2-7: block-major, one contiguous
            # whole-row DMA per block
            for tb in range(RAMP, NT):
                stage = stagep.tile([TBLK, OUTP], f32, tag="stage", bufs=2)
                for c in range(NCH):
                    mm_chunk(stage, c * MMN, tb, c // NMM, c % NMM)
                nc.sync.dma_start(
                    out[tb * TBLK:(tb + 1) * TBLK, :], stage[:, :SHARD]
                )

    nc.compile()
    return nc


def _densify(v1_weights, v1_rows, v1_cols, lm_weights, lm_rows, lm_cols):
    rows = np.concatenate([
        np.asarray(v1_rows).astype(np.int64),
        np.asarray(lm_rows).astype(np.int64) + NV1,
    ])
    cols = np.concatenate([
        np.asarray(v1_cols).astype(np.int64),
        np.asarray(lm_cols).astype(np.int64),
    ])
    w = np.concatenate([
        np.asarray(v1_weights, dtype=np.float32),
        np.asarray(lm_weights, dtype=np.float32),
    ])
    W = np.bincount(rows * NBKG + cols, weights=w, minlength=NPOST * NBKG)
    return W.astype(np.float32).reshape(NPOST, NBKG)


def kernel(rest, v1_weights, v1_rows, v1_cols, lm_weights, lm_rows, lm_cols):
    import ml_dtypes

    from concourse.bass_utils import run_bass_kernel_spmd

    bf16 = ml_dtypes.bfloat16

    global _compiled
    if _compiled is None:
        _compiled = _build_module()

    W = _densify(v1_weights, v1_rows, v1_cols, lm_weights, lm_rows, lm_cols)
    # 3-plane bf16 split: hi+mid+lo carries 24 mantissa bits == fp32-exact
    w_hi = W.astype(bf16)
    r1 = W - w_hi.astype(np.float32)
    w_mid = r1.astype(bf16)
    w_lo = (r1 - w_mid.astype(np.float32)).astype(bf16)

    rest32 = np.asarray(rest, np.float32)
    rest_b = rest32.astype(bf16)

    restT = np.zeros((KP, ROWS), bf16)
    restT[:NBKG, :B * T] = rest_b.T

    in_maps = []
    for c in range(NCORES):
        wpad = np.zeros((NPLANE, KP, OUTP), bf16)
        sl = slice(c * SHARD, (c + 1) * SHARD)
        wpad[0, :NBKG, :SHARD] = w_hi[sl].T
        wpad[1, :NBKG, :SHARD] = w_mid[sl].T
        wpad[2, :NBKG, :SHARD] = w_lo[sl].T
        in_maps.append({"restT": restT, "wT3": wpad})

    trace = bool(int(os.environ.get("KERNEL_TRACE", "0")))
    if trace:
        _install_ntff_shim()
    res = run_bass_kernel_spmd(
        _compiled, in_maps, core_ids=list(range(NCORES)), trace=trace
    )
    kernel.last_results = res
    full = np.concatenate(
        [res.results[c]["out"][:B * T, :] for c in range(NCORES)], axis=1
    )

    # sparse host correction for any rest value that bf16 can't represent
    # exactly (Poisson counts are small ints, so normally there are none)
    rest_err = rest32 - rest_b.astype(np.float32)
    if np.any(rest_err):
        ts, ks = np.nonzero(rest_err)
        for t, k in zip(ts, ks):
            full[t, :] += rest_err[t, k] * W[:, k]

    return full.reshape(B, T, NPOST)


def _install_ntff_shim():
    """The agent image's antenv lacks axon_hooks; register the NTFF profile
    hook by dlopening libaxon_pjrt.so directly (same path trn_boot uses)."""
    import sys
    import types

    if "antenv.axon_hooks" in sys.modules:
        return
    try:
        from trn_agent_boot.trn_boot import _ntff_profile_via_ctypes

        hook = _ntff_profile_via_ctypes("/opt/axon/libaxon_pjrt.so")
    except Exception:
        hook = None
    mod = types.ModuleType("antenv.axon_hooks")
    mod.get_axon_ntff_profile_hook = lambda: hook
    mod.set_axon_ntff_profile_hook = lambda h: None
    sys.modules["antenv.axon_hooks"] = mod
